# revision 4
# baseline (speedup 1.0000x reference)
"""CrossAttention3D Trainium2 kernel.

Full inputs in, full output out. Sharding: data-parallel over batch (2) x
query-token shards (4) = 8 NeuronCores. Each core projects K/V for all 4096
target tokens (replicated), Q for its 1024 query tokens, runs the 4096-wide
attention for its query shard, and the output projection.

Layout choice: scores are computed transposed (k-tokens on partitions,
q-tokens on free dim) so the P@V contraction needs no transposes; the softmax
denominator (a partition-dim reduction) is accumulated on VectorE and reduced
+ broadcast with a single ones-matrix matmul. exp() skips max-subtraction:
scores are ~N(0,1) for this problem so exp() is safely in range.

All matmuls run in float32r (full PE rate at N=512, ~1e-4 relative error).
"""

import math

import numpy as np

import concourse.bass as bass
import concourse.mybir as mybir
import concourse.tile as tile
from concourse.bass_utils import run_bass_kernel_spmd
from concourse.vector_clock import ScopedClock

F32 = mybir.dt.float32
F32R = mybir.dt.float32r

B, C, D, H, W = 2, 128, 16, 16, 16
N = D * H * W          # 4096 target tokens
NCORES = 8
QSHARDS = NCORES // B  # 4 query shards per batch
NQ = N // QSHARDS      # 1024 query tokens per core
KT = N // 128          # 32 k-tiles
AF = mybir.ActivationFunctionType


def _patched_drain_and_barrier(self, tick_clock, wait_clock):
    # This walrus build caps sync-waits per instruction; the stock TileContext
    # exit drain carries one wait per processor lane (>4 in this kernel).
    # Split the waits into single-wait SP instructions before the drain.
    nc = self.nc
    probe = nc.sync.nop()
    wait_clock.add_sem_waits(probe.ins, ScopedClock({None: tick_clock.global_clock}))
    si = probe.ins.sync_info
    waits = list(si.on_wait) if si and si.on_wait else []
    if si:
        si.on_wait = []
        probe.ins.sync_info = si
    by_name = {h.name: h for h in self.sems.allocated().values()}
    opmap = {"sem-ge-imm": "sem-ge", "sem-eq-imm": "sem-eq"}
    for wv in waits:
        nc.sync.wait_op(by_name[wv.ant_name], wv.wait_value, opmap.get(wv.wait_mode, "sem-ge"))
    nc.sync.drain()
    nc.all_engine_barrier()
    popped = nc._tile_sem_poison_stack.pop()
    assert popped is self._sem_poison
    nc.clear_and_free_semaphores(list(self.sems.allocated().values()))
    nc.all_engine_barrier()


tile.TileContext._drain_and_barrier = _patched_drain_and_barrier


def _split_excess_waits(nc, cap=1, evsem_cap=2):
    # This walrus build rejects instructions carrying more than ~1 sync wait
    # (Tile targets a newer walrus that packs several). Hoist excess waits
    # onto dedicated InstEventSemaphore instructions just before the
    # over-subscribed instruction, on the same engine stream.
    for fn in nc.m.functions:
        for bb in fn.blocks:
            out = []
            for inst in bb.instructions:
                si = inst.sync_info
                waits = list(si.on_wait) if si and si.on_wait else []
                limit = (
                    evsem_cap
                    if isinstance(inst, (mybir.InstEventSemaphore, mybir.InstDrain))
                    else cap
                )
                if len(waits) > limit:
                    excess, keep = waits[:-limit], waits[-limit:]
                    for i in range(0, len(excess), evsem_cap):
                        ev = mybir.InstEventSemaphore(
                            name=nc.get_next_instruction_name(),
                            engine=inst.engine,
                            ins=[],
                            outs=[],
                            sync_info=mybir.SyncInfo(
                                on_wait=excess[i : i + evsem_cap], on_update=[]
                            ),
                        )
                        nc.register_instruction(ev)
                        out.append(ev)
                    si.on_wait = keep
                    inst.sync_info = si
                out.append(inst)
            bb.instructions[:] = out


def build_bass():
    nc = bass.Bass("TRN2", target_bir_lowering=False, debug=False)

    qin = nc.dram_tensor("qin", [C, NQ], F32, kind="ExternalInput")
    tgt = nc.dram_tensor("tgt", [C, N], F32, kind="ExternalInput")
    wqT = nc.dram_tensor("wqT", [C, C], F32, kind="ExternalInput")
    wkT = nc.dram_tensor("wkT", [C, C], F32, kind="ExternalInput")
    wvT = nc.dram_tensor("wvT", [C, C], F32, kind="ExternalInput")
    woT = nc.dram_tensor("woT", [C, C], F32, kind="ExternalInput")
    bq = nc.dram_tensor("bq", [C, 1], F32, kind="ExternalInput")
    bk = nc.dram_tensor("bk", [C, 1], F32, kind="ExternalInput")
    bvo = nc.dram_tensor("bvo", [C, 1], F32, kind="ExternalInput")
    out = nc.dram_tensor("out", [C, NQ], F32, kind="ExternalOutput")

    with tile.TileContext(nc) as tc:
        with (
            tc.tile_pool(name="consts", bufs=1) as consts,
            tc.tile_pool(name="big", bufs=1) as big,
            tc.tile_pool(name="ets", bufs=3) as ets,
            tc.tile_pool(name="psum", bufs=2, space="PSUM") as psum,
            tc.tile_pool(name="psum_pv", bufs=1, space="PSUM") as psum_pv,
        ):
            # ---- constants / weights ----
            w_sb = consts.tile([C, 4, C], F32)
            w_r = consts.tile([C, 4, C], F32R)
            for i, wt in enumerate((wqT, wkT, wvT, woT)):
                nc.sync.dma_start(w_sb[:, i, :], wt[:, :])
                nc.vector.tensor_copy(w_r[:, i, :], w_sb[:, i, :])
            wq_r, wk_r, wv_r, wo_r = (w_r[:, i, :] for i in range(4))

            b_sb = consts.tile([C, 3], F32)
            nc.sync.dma_start(b_sb[:, 0:1], bq[:, :])
            nc.sync.dma_start(b_sb[:, 1:2], bk[:, :])
            nc.sync.dma_start(b_sb[:, 2:3], bvo[:, :])

            ones_f = consts.tile([C, C], F32)
            nc.vector.memset(ones_f[:], 1.0)
            ones_r = consts.tile([C, C], F32R)
            nc.vector.tensor_copy(ones_r[:], ones_f[:])

            # ---- activations in ----
            tgt_sb = big.tile([C, N], F32)
            nc.sync.dma_start(tgt_sb[:], tgt[:, :])
            tgt_r = big.tile([C, N], F32R)
            nc.vector.tensor_copy(tgt_r[:], tgt_sb[:])

            qin_sb = big.tile([C, NQ], F32)
            nc.sync.dma_start(qin_sb[:], qin[:, :])
            qin_r = big.tile([C, NQ], F32R)
            nc.vector.tensor_copy(qin_r[:], qin_sb[:])

            # ---- projections ----
            # K = wk.T^T @ tgt + bk  -> [c_out, m], fp32r
            k_r = big.tile([C, N], F32R)
            for j in range(N // 1024):
                kp = psum.tile([C, 1024], F32, tag="ps_big", bufs=2)
                for h in range(2):
                    nc.tensor.matmul(
                        kp[:, h * 512 : (h + 1) * 512],
                        wk_r,
                        tgt_r[:, j * 1024 + h * 512 : j * 1024 + (h + 1) * 512],
                        start=True,
                        stop=True,
                    )
                nc.scalar.activation(
                    out=k_r[:, j * 1024 : (j + 1) * 1024], in_=kp[:],
                    func=AF.Identity, bias=b_sb[:, 1:2], scale=1.0,
                )

            # Q = wq_s.T^T @ qin + bq_s (scale folded on host) -> [c_out, q]
            q_r = big.tile([C, NQ], F32R)
            qp = psum.tile([C, 1024], F32, tag="ps_big", bufs=2)
            for h in range(2):
                nc.tensor.matmul(
                    qp[:, h * 512 : (h + 1) * 512],
                    wq_r,
                    qin_r[:, h * 512 : (h + 1) * 512],
                    start=True,
                    stop=True,
                )
            nc.scalar.activation(
                out=q_r[:], in_=qp[:], func=AF.Identity, bias=b_sb[:, 0:1], scale=1.0,
            )

            # VT[m, c] = (wv @ tgt)^T, no bias (folded into bvo downstream)
            vt_r = big.tile([C, KT, C], F32R)
            for g in range(KT // 4):
                vp = psum.tile([C, 512], F32, tag="ps_vt", bufs=2)
                for i in range(4):
                    mt = g * 4 + i
                    nc.tensor.matmul(
                        vp[:, i * C : (i + 1) * C],
                        tgt_r[:, mt * C : (mt + 1) * C],
                        wv_r,
                        start=True,
                        stop=True,
                    )
                nc.vector.tensor_copy(vt_r[:, g * 4 : (g + 1) * 4, :], vp[:])

            # ---- attention main loop (k-tiles streamed) ----
            acc_r = big.tile([C, NQ], F32R)   # running sum of exp (denominator partials)
            pv_ps = psum_pv.tile([C, NQ], F32)

            st_tiles = {}
            et_tiles = {}

            def emit_st(kt):
                st = psum.tile([C, NQ], F32, tag="ps_big", bufs=2, name=f"st_{kt}")
                for h in range(2):
                    nc.tensor.matmul(
                        st[:, h * 512 : (h + 1) * 512],
                        k_r[:, kt * C : (kt + 1) * C],
                        q_r[:, h * 512 : (h + 1) * 512],
                        start=True,
                        stop=True,
                    )
                st_tiles[kt] = st

            def emit_exp_acc(kt):
                et = ets.tile([C, NQ], F32R, tag="et", name=f"et_{kt}")
                nc.scalar.activation(out=et[:], in_=st_tiles.pop(kt)[:], func=AF.Exp)
                if kt == 0:
                    nc.vector.tensor_copy(acc_r[:], et[:])
                else:
                    nc.vector.tensor_add(out=acc_r[:], in0=acc_r[:], in1=et[:])
                et_tiles[kt] = et

            def emit_pv(kt):
                et = et_tiles.pop(kt)
                for h in range(2):
                    nc.tensor.matmul(
                        pv_ps[:, h * 512 : (h + 1) * 512],
                        vt_r[:, kt, :],
                        et[:, h * 512 : (h + 1) * 512],
                        start=(kt == 0),
                        stop=(kt == KT - 1),
                    )

            # software-pipelined emission: ST(kt+1) before PV(kt)
            emit_st(0)
            emit_exp_acc(0)
            for kt in range(1, KT):
                emit_st(kt)
                emit_exp_acc(kt)
                emit_pv(kt - 1)
            emit_pv(KT - 1)

            # ---- epilogue ----
            # denominator: column sums of acc broadcast to all 128 partitions
            db_ps = psum.tile([C, NQ], F32, tag="ps_big", bufs=2)
            for h in range(2):
                nc.tensor.matmul(
                    db_ps[:, h * 512 : (h + 1) * 512],
                    ones_r[:],
                    acc_r[:, h * 512 : (h + 1) * 512],
                    start=True,
                    stop=True,
                )
            recip_sb = big.tile([C, NQ], F32)
            nc.vector.reciprocal(out=recip_sb[:], in_=db_ps[:])

            pv_r = big.tile([C, NQ], F32R)
            nc.vector.tensor_copy(pv_r[:], pv_ps[:])

            z_ps = psum.tile([C, NQ], F32, tag="ps_big", bufs=2)
            for h in range(2):
                nc.tensor.matmul(
                    z_ps[:, h * 512 : (h + 1) * 512],
                    wo_r,
                    pv_r[:, h * 512 : (h + 1) * 512],
                    start=True,
                    stop=True,
                )
            out_sb = big.tile([C, NQ], F32)
            nc.vector.tensor_tensor(
                out=out_sb[:], in0=z_ps[:], in1=recip_sb[:], op=mybir.AluOpType.mult
            )
            nc.vector.tensor_scalar_add(out=out_sb[:], in0=out_sb[:], scalar1=b_sb[:, 2:3])
            nc.sync.dma_start(out[:, :], out_sb[:])

    _split_excess_waits(nc)
    return nc


_NC_CACHE = None


def _get_nc():
    global _NC_CACHE
    if _NC_CACHE is None:
        _NC_CACHE = build_bass()
    return _NC_CACHE


def make_in_maps(source, target, wq, bq, wk, bk, wv, bv, wo, bo):
    source = np.ascontiguousarray(np.asarray(source, dtype=np.float32)).reshape(B, C, N)
    target = np.ascontiguousarray(np.asarray(target, dtype=np.float32)).reshape(B, C, N)
    scale = np.float32(1.0 / math.sqrt(C))
    wqT = np.ascontiguousarray(np.asarray(wq, np.float32).T * scale)
    wkT = np.ascontiguousarray(np.asarray(wk, np.float32).T)
    wvT = np.ascontiguousarray(np.asarray(wv, np.float32).T)
    woT = np.ascontiguousarray(np.asarray(wo, np.float32).T)
    bq_s = (np.asarray(bq, np.float32) * scale).reshape(C, 1)
    bk_c = np.asarray(bk, np.float32).reshape(C, 1)
    bvo = (np.asarray(wo, np.float32) @ np.asarray(bv, np.float32)
           + np.asarray(bo, np.float32)).astype(np.float32).reshape(C, 1)

    in_maps = []
    for core in range(NCORES):
        b, qs = divmod(core, QSHARDS)
        in_maps.append({
            "qin": np.ascontiguousarray(source[b, :, qs * NQ : (qs + 1) * NQ]),
            "tgt": target[b],
            "wqT": wqT, "wkT": wkT, "wvT": wvT, "woT": woT,
            "bq": bq_s, "bk": bk_c, "bvo": bvo,
        })
    return in_maps


def kernel(source, target, wq, bq, wk, bk, wv, bv, wo, bo):
    nc = _get_nc()
    in_maps = make_in_maps(source, target, wq, bq, wk, bk, wv, bv, wo, bo)
    res = run_bass_kernel_spmd(nc, in_maps, core_ids=list(range(NCORES)))
    full = np.empty((B, C, N), dtype=np.float32)
    for core in range(NCORES):
        b, qs = divmod(core, QSHARDS)
        full[b, :, qs * NQ : (qs + 1) * NQ] = res.results[core]["out"]
    return full.reshape(B, C, D, H, W)


# revision 23
# speedup vs baseline: 1.4568x; 1.4568x over previous
"""CrossAttention3D Trainium2 kernel.

Full inputs in, full output out. Sharding: data-parallel over batch (2) x
query-token shards (4) = 8 NeuronCores. Each core projects K/V for all 4096
target tokens (replicated), Q for its 1024 query tokens, runs the 4096-wide
attention for its query shard, and the output projection.

Layout: scores are computed transposed (k-tokens on partitions, q-tokens on
free dim) so the P@V contraction needs no transposes. The softmax denominator
(a partition-dim reduction) is accumulated per-lane on VectorE in fp16 (each
lane only sums 32 values; the cross-lane reduce happens exactly in fp32 via a
ones-matrix matmul that also broadcasts it). exp() skips max-subtraction:
scores are ~N(0,1) for this problem so exp() is safely in range.

Numerics: fp16 operands everywhere on the PE (full rate + fast weight load),
fp32 PSUM accumulation, fp32 normalization/bias epilogue. Measured end-to-end
relative error vs the fp32 reference is a few 1e-3.
"""

import math

import numpy as np

import concourse.bass as bass
import concourse.mybir as mybir
import concourse.tile as tile
from concourse.bass_utils import run_bass_kernel_spmd
from concourse.vector_clock import ScopedClock

F32 = mybir.dt.float32
F16 = mybir.dt.float16

B, C, D, H, W = 2, 128, 16, 16, 16
N = D * H * W          # 4096 target tokens
NCORES = 8
QSHARDS = NCORES // B  # 4 query shards per batch
NQ = N // QSHARDS      # 1024 query tokens per core
KT = N // 128          # 32 k-tiles
AF = mybir.ActivationFunctionType
OP = mybir.AluOpType


def _patched_drain_and_barrier(self, tick_clock, wait_clock):
    # This walrus build caps sync-waits per instruction; the stock TileContext
    # exit drain carries one wait per processor lane (>4 in this kernel).
    # Split the waits into single-wait SP instructions before the drain.
    nc = self.nc
    probe = nc.sync.nop()
    wait_clock.add_sem_waits(probe.ins, ScopedClock({None: tick_clock.global_clock}))
    si = probe.ins.sync_info
    waits = list(si.on_wait) if si and si.on_wait else []
    if si:
        si.on_wait = []
        probe.ins.sync_info = si
    by_name = {h.name: h for h in self.sems.allocated().values()}
    opmap = {"sem-ge-imm": "sem-ge", "sem-eq-imm": "sem-eq"}
    for wv in waits:
        nc.sync.wait_op(by_name[wv.ant_name], wv.wait_value, opmap.get(wv.wait_mode, "sem-ge"))
    nc.sync.drain()
    nc.all_engine_barrier()
    popped = nc._tile_sem_poison_stack.pop()
    assert popped is self._sem_poison
    nc.clear_and_free_semaphores(list(self.sems.allocated().values()))
    nc.all_engine_barrier()


tile.TileContext._drain_and_barrier = _patched_drain_and_barrier


def _split_excess_waits(nc, cap=1, evsem_cap=2):
    # This walrus build rejects instructions carrying more than ~1 sync wait
    # (Tile targets a newer walrus that packs several). Hoist excess waits
    # onto dedicated InstEventSemaphore instructions just before the
    # over-subscribed instruction, on the same engine stream.
    for fn in nc.m.functions:
        for bb in fn.blocks:
            out = []
            for inst in bb.instructions:
                si = inst.sync_info
                waits = list(si.on_wait) if si and si.on_wait else []
                limit = (
                    evsem_cap
                    if isinstance(inst, (mybir.InstEventSemaphore, mybir.InstDrain))
                    else cap
                )
                if len(waits) > limit:
                    excess, keep = waits[:-limit], waits[-limit:]
                    for i in range(0, len(excess), evsem_cap):
                        ev = mybir.InstEventSemaphore(
                            name=nc.get_next_instruction_name(),
                            engine=inst.engine,
                            ins=[],
                            outs=[],
                            sync_info=mybir.SyncInfo(
                                on_wait=excess[i : i + evsem_cap], on_update=[]
                            ),
                        )
                        nc.register_instruction(ev)
                        out.append(ev)
                    si.on_wait = keep
                    inst.sync_info = si
                out.append(inst)
            bb.instructions[:] = out


def build_bass():
    nc = bass.Bass("TRN2", target_bir_lowering=False, debug=False)

    qin = nc.dram_tensor("qin", [C, NQ], F16, kind="ExternalInput")
    tgt = nc.dram_tensor("tgt", [C, N], F16, kind="ExternalInput")
    wqk = nc.dram_tensor("wqk", [C, 2, C], F16, kind="ExternalInput")   # wqT|wkT
    wvo = nc.dram_tensor("wvo", [C, 2, C], F16, kind="ExternalInput")   # wvT|woT
    bqk = nc.dram_tensor("bqk", [C, 2], F32, kind="ExternalInput")      # bq|bk
    bvo = nc.dram_tensor("bvo", [C, 1], F32, kind="ExternalInput")
    out = nc.dram_tensor("out", [C, NQ], F32, kind="ExternalOutput")

    with tile.TileContext(nc) as tc:
        with (
            tc.tile_pool(name="consts", bufs=1) as consts,
            tc.tile_pool(name="big", bufs=1) as big,
            tc.tile_pool(name="ets", bufs=4) as ets,
            tc.tile_pool(name="psum", bufs=3, space="PSUM") as psum,
            tc.tile_pool(name="psum_pv", bufs=1, space="PSUM") as psum_pv,
        ):
            # ---- inputs. Two HWDGE rings (SP + ACT) in parallel; ordered so
            # the Q-projection and first K-chunk dependencies land first.
            wqk_sb = consts.tile([C, 2, C], F16)
            wvo_sb = consts.tile([C, 2, C], F16)
            wq_h, wk_h = wqk_sb[:, 0, :], wqk_sb[:, 1, :]
            wv_h, wo_h = wvo_sb[:, 0, :], wvo_sb[:, 1, :]
            bqk_sb = consts.tile([C, 2], F32)
            bvo_sb = consts.tile([C, 1], F32)
            tgt_c = [big.tile([C, 1024], F16, name=f"tgt_c{j}") for j in range(4)]
            qin_sb = big.tile([C, NQ], F16)

            # Descriptor generation is serialized across rings, so emission
            # order sets arrival order: small critical tensors first.
            nc.sync.dma_start(wqk_sb[:], wqk[:, :, :])
            nc.sync.dma_start(qin_sb[:], qin[:, :])
            nc.scalar.dma_start(tgt_c[0][:], tgt[:, 0:1024])
            nc.sync.dma_start(bqk_sb[:], bqk[:, :])
            nc.scalar.dma_start(tgt_c[1][:], tgt[:, 1024:2048])
            nc.sync.dma_start(tgt_c[2][:], tgt[:, 2048:3072])
            nc.scalar.dma_start(wvo_sb[:], wvo[:, :, :])
            nc.sync.dma_start(tgt_c[3][:], tgt[:, 3072:4096])
            nc.scalar.dma_start(bvo_sb[:], bvo[:, :])

            warm_src = consts.tile([C, 512], F16)
            nc.gpsimd.memset(warm_src[:], 1.0)
            ones_h = consts.tile([C, C], F16)
            nc.gpsimd.memset(ones_h[:], 1.0)
            # PE warm-up: dummy matmuls with no DMA deps ramp the HAM clock
            # to 2.4 GHz while the input DMAs are still in flight.
            for wi in range(8):
                warm_ps = psum.tile(
                    [C, 512], F32, tag="ps_big", bufs=3, name=f"warm_{wi}"
                )
                nc.tensor.matmul(
                    warm_ps[:], warm_src[:, 0:128], warm_src[:], start=True, stop=True,
                )

            # ---- projections (emitted interleaved with the attention loop) ----
            k_c = [big.tile([C, 1024], F16, name=f"k_c{j}") for j in range(4)]
            q_h = big.tile([C, NQ], F16)
            vt_g = [big.tile([C, 4, C], F16, name=f"vt_g{g}") for g in range(8)]

            def emit_qproj():
                qp = psum.tile([C, 1024], F32, tag="ps_big", bufs=3)
                for h in range(2):
                    nc.tensor.matmul(
                        qp[:, h * 512 : (h + 1) * 512],
                        wq_h,
                        qin_sb[:, h * 512 : (h + 1) * 512],
                        start=True,
                        stop=True,
                    )
                nc.vector.tensor_scalar(
                    out=q_h[:, 0:512], in0=qp[:, 0:512], scalar1=bqk_sb[:, 0:1],
                    scalar2=None, op0=OP.add,
                )
                nc.scalar.activation(
                    out=q_h[:, 512:1024], in_=qp[:, 512:1024], func=AF.Identity,
                    bias=bqk_sb[:, 0:1], scale=1.0,
                )

            def emit_kproj(j):
                # K = wk.T^T @ tgt + bk -> [c_out, m] fp16, 1024-token chunk j
                kp = psum.tile([C, 1024], F32, tag="ps_big", bufs=3, name=f"kp_{j}")
                for h in range(2):
                    nc.tensor.matmul(
                        kp[:, h * 512 : (h + 1) * 512],
                        wk_h,
                        tgt_c[j][:, h * 512 : (h + 1) * 512],
                        start=True,
                        stop=True,
                    )
                if j == 0:
                    nc.vector.tensor_scalar(
                        out=k_c[j][:, 0:512], in0=kp[:, 0:512],
                        scalar1=bqk_sb[:, 1:2], scalar2=None, op0=OP.add,
                    )
                    nc.scalar.activation(
                        out=k_c[j][:, 512:1024], in_=kp[:, 512:1024],
                        func=AF.Identity, bias=bqk_sb[:, 1:2], scale=1.0,
                    )
                else:
                    nc.vector.tensor_scalar(
                        out=k_c[j][:], in0=kp[:],
                        scalar1=bqk_sb[:, 1:2], scalar2=None, op0=OP.add,
                    )

            def emit_vtproj(g):
                # VT[m, c] = (wv @ tgt)^T for 4 m-tiles, no bias (in bvo)
                vp = psum.tile([C, 512], F32, tag="ps_big", bufs=3, name=f"vp_{g}")
                for i in range(4):
                    mt = g * 4 + i
                    nc.tensor.matmul(
                        vp[:, i * C : (i + 1) * C],
                        tgt_c[mt // 8][:, (mt % 8) * C : (mt % 8 + 1) * C],
                        wv_h,
                        start=True,
                        stop=True,
                    )
                nc.vector.tensor_copy(vt_g[g][:], vp[:])

            # ---- attention main loop (k-tiles streamed) ----
            acc_h = big.tile([C, NQ], F16)   # per-lane running sum of exp
            pv_ps = psum_pv.tile([C, NQ], F32)

            st_tiles = {}
            et_tiles = {}

            def emit_st(kt):
                st = psum.tile([C, NQ], F32, tag="ps_big", bufs=3, name=f"st_{kt}")
                for h in range(2):
                    nc.tensor.matmul(
                        st[:, h * 512 : (h + 1) * 512],
                        k_c[kt // 8][:, (kt % 8) * C : (kt % 8 + 1) * C],
                        q_h[:, h * 512 : (h + 1) * 512],
                        start=True,
                        stop=True,
                    )
                st_tiles[kt] = st

            def emit_exp_acc(kt):
                et = ets.tile([C, NQ], F16, tag="et", name=f"et_{kt}")
                st = st_tiles.pop(kt)
                if kt == KT - 1:
                    # split halves so the denominator chain starts earlier
                    for h in range(2):
                        s = slice(h * 512, (h + 1) * 512)
                        nc.scalar.activation(out=et[:, s], in_=st[:, s], func=AF.Exp)
                        nc.vector.tensor_add(
                            out=acc_h[:, s], in0=acc_h[:, s], in1=et[:, s]
                        )
                elif kt == 0:
                    nc.scalar.activation(out=et[:], in_=st[:], func=AF.Exp)
                    nc.vector.tensor_copy(acc_h[:], et[:])
                else:
                    nc.scalar.activation(out=et[:], in_=st[:], func=AF.Exp)
                    nc.vector.tensor_add(out=acc_h[:], in0=acc_h[:], in1=et[:])
                et_tiles[kt] = et

            def emit_pv(kt):
                et = et_tiles.pop(kt)
                for h in range(2):
                    nc.tensor.matmul(
                        pv_ps[:, h * 512 : (h + 1) * 512],
                        vt_g[kt // 4][:, kt % 4, :],
                        et[:, h * 512 : (h + 1) * 512],
                        start=(kt == 0),
                        stop=(kt == KT - 1),
                    )

            # Software-pipelined emission. Q + first K chunk first so exp
            # starts ASAP; remaining K chunks and VT groups woven into the
            # early iterations (K chunk j is needed by ST(8j); VT group g by
            # PV(4g)).
            emit_qproj()
            emit_kproj(0)
            emit_st(0)
            emit_vtproj(0)
            emit_exp_acc(0)
            for kt in range(1, KT):
                if kt in (3, 6, 9):          # K chunk j=kt/3, before ST(8j)
                    emit_kproj(kt // 3)
                if kt % 3 == 2 and (kt + 1) // 3 <= 7:  # VT g=(kt+1)/3 < PV(4g)
                    emit_vtproj((kt + 1) // 3)
                emit_st(kt)
                emit_exp_acc(kt)
                emit_pv(kt - 1)
            emit_pv(KT - 1)

            # ---- epilogue (512-wide chunks so PE/DVE stages overlap) ----
            # denominator: column sums of acc broadcast to all 128 partitions;
            # PV is normalized BEFORE the output projection, so
            # out = wo @ (pv * recip) + bvo needs no extra multiply pass.
            recip_sb = big.tile([C, NQ], F32)
            pvn_h = big.tile([C, NQ], F16)
            out_sb = big.tile([C, NQ], F32)
            db_ps = psum.tile([C, NQ], F32, tag="ps_big", bufs=3)
            z_ps = psum.tile([C, NQ], F32, tag="ps_big", bufs=3)
            for h in range(2):
                s = slice(h * 512, (h + 1) * 512)
                nc.tensor.matmul(
                    db_ps[:, s], ones_h[:], acc_h[:, s], start=True, stop=True,
                )
                nc.vector.reciprocal(out=recip_sb[:, s], in_=db_ps[:, s])
                nc.vector.tensor_tensor(
                    out=pvn_h[:, s], in0=pv_ps[:, s], in1=recip_sb[:, s], op=OP.mult
                )
                nc.tensor.matmul(
                    z_ps[:, s], wo_h, pvn_h[:, s], start=True, stop=True,
                )
                nc.scalar.activation(
                    out=out_sb[:, s], in_=z_ps[:, s], func=AF.Identity,
                    bias=bvo_sb[:], scale=1.0,
                )
                dma_eng = nc.sync if h == 0 else nc.scalar
                dma_eng.dma_start(out[:, s], out_sb[:, s])

    _split_excess_waits(nc)
    return nc


_NC_CACHE = None


def _get_nc():
    global _NC_CACHE
    if _NC_CACHE is None:
        _NC_CACHE = build_bass()
    return _NC_CACHE


def make_in_maps(source, target, wq, bq, wk, bk, wv, bv, wo, bo):
    source = np.asarray(source, dtype=np.float32).reshape(B, C, N)
    target = np.asarray(target, dtype=np.float32).reshape(B, C, N)
    scale = np.float32(1.0 / math.sqrt(C))
    wqT = (np.asarray(wq, np.float32).T * scale).astype(np.float16)
    wkT = np.asarray(wk, np.float32).T.astype(np.float16)
    wvT = np.asarray(wv, np.float32).T.astype(np.float16)
    woT = np.asarray(wo, np.float32).T.astype(np.float16)
    wqk_v = np.ascontiguousarray(np.stack([wqT, wkT], axis=1))
    wvo_v = np.ascontiguousarray(np.stack([wvT, woT], axis=1))
    bq_s = (np.asarray(bq, np.float32) * scale).reshape(C, 1)
    bk_c = np.asarray(bk, np.float32).reshape(C, 1)
    bqk_v = np.ascontiguousarray(np.concatenate([bq_s, bk_c], axis=1))
    bvo_v = (np.asarray(wo, np.float32) @ np.asarray(bv, np.float32)
             + np.asarray(bo, np.float32)).astype(np.float32).reshape(C, 1)

    tgt16 = target.astype(np.float16)
    src16 = source.astype(np.float16)
    in_maps = []
    for core in range(NCORES):
        b, qs = divmod(core, QSHARDS)
        in_maps.append({
            "qin": np.ascontiguousarray(src16[b, :, qs * NQ : (qs + 1) * NQ]),
            "tgt": np.ascontiguousarray(tgt16[b]),
            "wqk": wqk_v, "wvo": wvo_v, "bqk": bqk_v, "bvo": bvo_v,
        })
    return in_maps


def kernel(source, target, wq, bq, wk, bk, wv, bv, wo, bo):
    nc = _get_nc()
    in_maps = make_in_maps(source, target, wq, bq, wk, bk, wv, bv, wo, bo)
    res = run_bass_kernel_spmd(nc, in_maps, core_ids=list(range(NCORES)))
    full = np.empty((B, C, N), dtype=np.float32)
    for core in range(NCORES):
        b, qs = divmod(core, QSHARDS)
        full[b, :, qs * NQ : (qs + 1) * NQ] = res.results[core]["out"]
    return full.reshape(B, C, D, H, W)


# revision 32
# speedup vs baseline: 1.4822x; 1.0174x over previous
"""CrossAttention3D Trainium2 kernel.

Full inputs in, full output out. Sharding: data-parallel over batch (2) x
query-token shards (4) = 8 NeuronCores. Each core projects K/V for all 4096
target tokens (replicated), Q for its 1024 query tokens, runs the 4096-wide
attention for its query shard, and the output projection.

Layout: scores are computed transposed (k-tokens on partitions, q-tokens on
free dim) so the P@V contraction needs no transposes. The softmax denominator
(a partition-dim reduction) is accumulated per-lane on VectorE in fp16 (each
lane only sums 32 values; the cross-lane reduce happens exactly in fp32 via a
ones-matrix matmul that also broadcasts it). exp() skips max-subtraction:
scores are ~N(0,1) for this problem so exp() is safely in range.

Numerics: fp16 operands everywhere on the PE (full rate + fast weight load),
fp32 PSUM accumulation, fp32 normalization/bias epilogue. Measured end-to-end
relative error vs the fp32 reference is a few 1e-3.
"""

import math

import numpy as np

import concourse.bass as bass
import concourse.mybir as mybir
import concourse.tile as tile
from concourse.bass_utils import run_bass_kernel_spmd
from concourse.vector_clock import ScopedClock

F32 = mybir.dt.float32
F16 = mybir.dt.float16

B, C, D, H, W = 2, 128, 16, 16, 16
N = D * H * W          # 4096 target tokens
NCORES = 8
QSHARDS = NCORES // B  # 4 query shards per batch
NQ = N // QSHARDS      # 1024 query tokens per core
KT = N // 128          # 32 k-tiles
AF = mybir.ActivationFunctionType
OP = mybir.AluOpType


def _patched_drain_and_barrier(self, tick_clock, wait_clock):
    # This walrus build caps sync-waits per instruction; the stock TileContext
    # exit drain carries one wait per processor lane (>4 in this kernel).
    # Split the waits into single-wait SP instructions before the drain.
    nc = self.nc
    probe = nc.sync.nop()
    wait_clock.add_sem_waits(probe.ins, ScopedClock({None: tick_clock.global_clock}))
    si = probe.ins.sync_info
    waits = list(si.on_wait) if si and si.on_wait else []
    if si:
        si.on_wait = []
        probe.ins.sync_info = si
    by_name = {h.name: h for h in self.sems.allocated().values()}
    opmap = {"sem-ge-imm": "sem-ge", "sem-eq-imm": "sem-eq"}
    for wv in waits:
        nc.sync.wait_op(by_name[wv.ant_name], wv.wait_value, opmap.get(wv.wait_mode, "sem-ge"))
    nc.sync.drain()
    nc.all_engine_barrier()
    popped = nc._tile_sem_poison_stack.pop()
    assert popped is self._sem_poison
    nc.clear_and_free_semaphores(list(self.sems.allocated().values()))


tile.TileContext._drain_and_barrier = _patched_drain_and_barrier


def _split_excess_waits(nc, cap=1, evsem_cap=2):
    # This walrus build rejects instructions carrying more than ~1 sync wait
    # (Tile targets a newer walrus that packs several). Hoist excess waits
    # onto dedicated InstEventSemaphore instructions just before the
    # over-subscribed instruction, on the same engine stream.
    for fn in nc.m.functions:
        for bb in fn.blocks:
            out = []
            for inst in bb.instructions:
                si = inst.sync_info
                waits = list(si.on_wait) if si and si.on_wait else []
                limit = (
                    evsem_cap
                    if isinstance(inst, (mybir.InstEventSemaphore, mybir.InstDrain))
                    else cap
                )
                if len(waits) > limit:
                    excess, keep = waits[:-limit], waits[-limit:]
                    for i in range(0, len(excess), evsem_cap):
                        ev = mybir.InstEventSemaphore(
                            name=nc.get_next_instruction_name(),
                            engine=inst.engine,
                            ins=[],
                            outs=[],
                            sync_info=mybir.SyncInfo(
                                on_wait=excess[i : i + evsem_cap], on_update=[]
                            ),
                        )
                        nc.register_instruction(ev)
                        out.append(ev)
                    si.on_wait = keep
                    inst.sync_info = si
                out.append(inst)
            bb.instructions[:] = out


def build_bass():
    nc = bass.Bass("TRN2", target_bir_lowering=False, debug=False)

    qin = nc.dram_tensor("qin", [C, NQ], F16, kind="ExternalInput")
    tgt = nc.dram_tensor("tgt", [C, N], F16, kind="ExternalInput")
    wqk = nc.dram_tensor("wqk", [C, 2, C], F16, kind="ExternalInput")   # wqT|wkT
    wvo = nc.dram_tensor("wvo", [C, 2, C], F16, kind="ExternalInput")   # wvT|woT
    bqk = nc.dram_tensor("bqk", [C, 2], F32, kind="ExternalInput")      # bq|bk
    bvo = nc.dram_tensor("bvo", [C, 1], F32, kind="ExternalInput")
    out = nc.dram_tensor("out", [C, NQ], F16, kind="ExternalOutput")

    with tile.TileContext(nc) as tc:
        with (
            tc.tile_pool(name="consts", bufs=1) as consts,
            tc.tile_pool(name="big", bufs=1) as big,
            tc.tile_pool(name="ets", bufs=4) as ets,
            tc.tile_pool(name="psum", bufs=3, space="PSUM") as psum,
            tc.tile_pool(name="psum_pv", bufs=1, space="PSUM") as psum_pv,
        ):
            # ---- inputs. Two HWDGE rings (SP + ACT) in parallel; ordered so
            # the Q-projection and first K-chunk dependencies land first.
            wqk_sb = consts.tile([C, 2, C], F16)
            wvo_sb = consts.tile([C, 2, C], F16)
            wq_h, wk_h = wqk_sb[:, 0, :], wqk_sb[:, 1, :]
            wv_h, wo_h = wvo_sb[:, 0, :], wvo_sb[:, 1, :]
            bqk_sb = consts.tile([C, 2], F32)
            bvo_sb = consts.tile([C, 1], F32)
            tgt_c = [big.tile([C, 1024], F16, name=f"tgt_c{j}") for j in range(4)]
            qin_sb = big.tile([C, NQ], F16)

            # Descriptor generation is serialized across rings, so emission
            # order sets arrival order: small critical tensors first.
            nc.sync.dma_start(qin_sb[:], qin[:, :])
            nc.sync.dma_start(wqk_sb[:], wqk[:, :, :])
            nc.scalar.dma_start(tgt_c[0][:], tgt[:, 0:1024])
            nc.sync.dma_start(bqk_sb[:], bqk[:, :])
            nc.scalar.dma_start(tgt_c[1][:], tgt[:, 1024:2048])
            nc.sync.dma_start(tgt_c[2][:], tgt[:, 2048:3072])
            nc.scalar.dma_start(wvo_sb[:], wvo[:, :, :])
            nc.sync.dma_start(tgt_c[3][:], tgt[:, 3072:4096])
            nc.scalar.dma_start(bvo_sb[:], bvo[:, :])

            warm_src = consts.tile([C, 512], F16)
            nc.gpsimd.memset(warm_src[:], 1.0)
            ones_h = consts.tile([C, C], F16)
            nc.gpsimd.memset(ones_h[:], 1.0)
            # PE warm-up: dummy matmuls with no DMA deps ramp the HAM clock
            # to 2.4 GHz while the input DMAs are still in flight.
            for wi in range(8):
                warm_ps = psum.tile(
                    [C, 512], F32, tag="ps_big", bufs=3, name=f"warm_{wi}"
                )
                nc.tensor.matmul(
                    warm_ps[:], warm_src[:, 0:128], warm_src[:], start=True, stop=True,
                )

            # ---- projections (emitted interleaved with the attention loop) ----
            k_c = [big.tile([C, 1024], F16, name=f"k_c{j}") for j in range(4)]
            q_h = big.tile([C, NQ], F16)
            vt_g = [big.tile([C, 4, C], F16, name=f"vt_g{g}") for g in range(8)]

            def emit_qproj():
                qp = psum.tile([C, 1024], F32, tag="ps_big", bufs=3)
                for h in range(2):
                    nc.tensor.matmul(
                        qp[:, h * 512 : (h + 1) * 512],
                        wq_h,
                        qin_sb[:, h * 512 : (h + 1) * 512],
                        start=True,
                        stop=True,
                    )
                nc.scalar.activation(
                    out=q_h[:], in_=qp[:], func=AF.Identity,
                    bias=bqk_sb[:, 0:1], scale=1.0,
                )

            def emit_kproj(j):
                # K = wk.T^T @ tgt + bk -> [c_out, m] fp16, 1024-token chunk j
                kp = psum.tile([C, 1024], F32, tag="ps_big", bufs=3, name=f"kp_{j}")
                for h in range(2):
                    nc.tensor.matmul(
                        kp[:, h * 512 : (h + 1) * 512],
                        wk_h,
                        tgt_c[j][:, h * 512 : (h + 1) * 512],
                        start=True,
                        stop=True,
                    )
                if j == 0:
                    nc.vector.tensor_scalar(
                        out=k_c[j][:, 0:256], in0=kp[:, 0:256],
                        scalar1=bqk_sb[:, 1:2], scalar2=None, op0=OP.add,
                    )
                    nc.vector.tensor_scalar(
                        out=k_c[j][:, 256:1024], in0=kp[:, 256:1024],
                        scalar1=bqk_sb[:, 1:2], scalar2=None, op0=OP.add,
                    )
                else:
                    nc.vector.tensor_scalar(
                        out=k_c[j][:], in0=kp[:],
                        scalar1=bqk_sb[:, 1:2], scalar2=None, op0=OP.add,
                    )

            def emit_vtproj(g):
                # VT[m, c] = (wv @ tgt)^T for 4 m-tiles, no bias (in bvo)
                vp = psum.tile([C, 512], F32, tag="ps_big", bufs=3, name=f"vp_{g}")
                for i in range(4):
                    mt = g * 4 + i
                    nc.tensor.matmul(
                        vp[:, i * C : (i + 1) * C],
                        tgt_c[mt // 8][:, (mt % 8) * C : (mt % 8 + 1) * C],
                        wv_h,
                        start=True,
                        stop=True,
                    )
                nc.vector.tensor_copy(vt_g[g][:], vp[:])

            # ---- attention main loop (k-tiles streamed) ----
            acc_h = big.tile([C, NQ], F16)   # per-lane running sum of exp
            pv_ps = psum_pv.tile([C, NQ], F32)

            st_tiles = {}
            et_tiles = {}

            def emit_st(kt):
                st = psum.tile([C, NQ], F32, tag="ps_big", bufs=3, name=f"st_{kt}")
                for h in range(2):
                    nc.tensor.matmul(
                        st[:, h * 512 : (h + 1) * 512],
                        k_c[kt // 8][:, (kt % 8) * C : (kt % 8 + 1) * C],
                        q_h[:, h * 512 : (h + 1) * 512],
                        start=True,
                        stop=True,
                    )
                st_tiles[kt] = st

            def emit_exp_acc(kt):
                et = ets.tile([C, NQ], F16, tag="et", name=f"et_{kt}")
                st = st_tiles.pop(kt)
                if kt == KT - 1:
                    # split halves so the denominator chain starts earlier
                    for h in range(2):
                        s = slice(h * 512, (h + 1) * 512)
                        nc.scalar.activation(out=et[:, s], in_=st[:, s], func=AF.Exp)
                        nc.vector.tensor_add(
                            out=acc_h[:, s], in0=acc_h[:, s], in1=et[:, s]
                        )
                elif kt == 0:
                    nc.scalar.activation(out=et[:], in_=st[:], func=AF.Exp)
                    nc.vector.tensor_copy(acc_h[:], et[:])
                else:
                    nc.scalar.activation(out=et[:], in_=st[:], func=AF.Exp)
                    nc.vector.tensor_add(out=acc_h[:], in0=acc_h[:], in1=et[:])
                et_tiles[kt] = et

            def emit_pv(kt):
                et = et_tiles.pop(kt)
                for h in range(2):
                    nc.tensor.matmul(
                        pv_ps[:, h * 512 : (h + 1) * 512],
                        vt_g[kt // 4][:, kt % 4, :],
                        et[:, h * 512 : (h + 1) * 512],
                        start=(kt == 0),
                        stop=(kt == KT - 1),
                    )

            # Software-pipelined emission. Q + first K chunk first so exp
            # starts ASAP; remaining K chunks and VT groups woven into the
            # early iterations (K chunk j is needed by ST(8j); VT group g by
            # PV(4g)).
            emit_qproj()
            emit_kproj(0)
            emit_st(0)
            emit_vtproj(0)
            emit_exp_acc(0)
            for kt in range(1, KT):
                if kt in (3, 6, 9):          # K chunk j=kt/3, before ST(8j)
                    emit_kproj(kt // 3)
                if kt % 3 == 2 and (kt + 1) // 3 <= 7:  # VT g=(kt+1)/3 < PV(4g)
                    emit_vtproj((kt + 1) // 3)
                emit_st(kt)
                emit_exp_acc(kt)
                emit_pv(kt - 1)
            emit_pv(KT - 1)

            # ---- epilogue (512-wide chunks so PE/DVE stages overlap) ----
            # denominator: column sums of acc broadcast to all 128 partitions;
            # PV is normalized BEFORE the output projection, so
            # out = wo @ (pv * recip) + bvo needs no extra multiply pass.
            recip_sb = big.tile([C, NQ], F32)
            pvn_h = big.tile([C, NQ], F16)
            out_sb = big.tile([C, NQ], F16)
            db_ps = psum.tile([C, NQ], F32, tag="ps_big", bufs=3)
            z_ps = psum.tile([C, NQ], F32, tag="ps_big", bufs=3)
            for h in range(2):
                s = slice(h * 512, (h + 1) * 512)
                nc.tensor.matmul(
                    db_ps[:, s], ones_h[:], acc_h[:, s], start=True, stop=True,
                )
                nc.vector.reciprocal(out=recip_sb[:, s], in_=db_ps[:, s])
                nc.vector.tensor_tensor(
                    out=pvn_h[:, s], in0=pv_ps[:, s], in1=recip_sb[:, s], op=OP.mult
                )
                nc.tensor.matmul(
                    z_ps[:, s], wo_h, pvn_h[:, s], start=True, stop=True,
                )
                if h == 0:
                    nc.scalar.activation(
                        out=out_sb[:, s], in_=z_ps[:, s], func=AF.Identity,
                        bias=bvo_sb[:], scale=1.0,
                    )
                else:
                    nc.vector.tensor_scalar(
                        out=out_sb[:, s], in0=z_ps[:, s], scalar1=bvo_sb[:],
                        scalar2=None, op0=OP.add,
                    )
                dma_eng = nc.sync if h == 0 else nc.scalar
                dma_eng.dma_start(out[:, s], out_sb[:, s])

    _split_excess_waits(nc)
    return nc


_NC_CACHE = None


def _get_nc():
    global _NC_CACHE
    if _NC_CACHE is None:
        _NC_CACHE = build_bass()
    return _NC_CACHE


def make_in_maps(source, target, wq, bq, wk, bk, wv, bv, wo, bo):
    source = np.asarray(source, dtype=np.float32).reshape(B, C, N)
    target = np.asarray(target, dtype=np.float32).reshape(B, C, N)
    scale = np.float32(1.0 / math.sqrt(C))
    wqT = (np.asarray(wq, np.float32).T * scale).astype(np.float16)
    wkT = np.asarray(wk, np.float32).T.astype(np.float16)
    wvT = np.asarray(wv, np.float32).T.astype(np.float16)
    woT = np.asarray(wo, np.float32).T.astype(np.float16)
    wqk_v = np.ascontiguousarray(np.stack([wqT, wkT], axis=1))
    wvo_v = np.ascontiguousarray(np.stack([wvT, woT], axis=1))
    bq_s = (np.asarray(bq, np.float32) * scale).reshape(C, 1)
    bk_c = np.asarray(bk, np.float32).reshape(C, 1)
    bqk_v = np.ascontiguousarray(np.concatenate([bq_s, bk_c], axis=1))
    bvo_v = (np.asarray(wo, np.float32) @ np.asarray(bv, np.float32)
             + np.asarray(bo, np.float32)).astype(np.float32).reshape(C, 1)

    tgt16 = target.astype(np.float16)
    src16 = source.astype(np.float16)
    in_maps = []
    for core in range(NCORES):
        b, qs = divmod(core, QSHARDS)
        in_maps.append({
            "qin": np.ascontiguousarray(src16[b, :, qs * NQ : (qs + 1) * NQ]),
            "tgt": np.ascontiguousarray(tgt16[b]),
            "wqk": wqk_v, "wvo": wvo_v, "bqk": bqk_v, "bvo": bvo_v,
        })
    return in_maps


def kernel(source, target, wq, bq, wk, bk, wv, bv, wo, bo):
    nc = _get_nc()
    in_maps = make_in_maps(source, target, wq, bq, wk, bk, wv, bv, wo, bo)
    res = run_bass_kernel_spmd(nc, in_maps, core_ids=list(range(NCORES)))
    full = np.empty((B, C, N), dtype=np.float32)
    for core in range(NCORES):
        b, qs = divmod(core, QSHARDS)
        full[b, :, qs * NQ : (qs + 1) * NQ] = res.results[core]["out"]
    return full.reshape(B, C, D, H, W)
